# revision 80
# baseline (speedup 1.0000x reference)
"""Trainium2 Bass kernel for DualHazardHead (moe_routing).

Computation per token t:
  x = concat(h, a, d, age)            [594]
  z = gelu(x @ Wt + bt)               [256]
  pw = softmax(h @ Wr + br)           [7]
  inst  = z @ Wbi + bbi + sum_p pw_p (z @ Wei_p + bei_p)   [20]
  group = z @ Wbg + bbg + sum_p pw_p (z @ Weg_p + beg_p)   [20]

Sharding: pure data parallel over B (32 -> 4 per core) on 8 NeuronCores.

v2 design (per core, NTOK=8192 tokens, 16 macro tiles of 512):
  - x is transposed to feature-major ON HOST: xT[m, k, b, j] bf16 with
    k=feat%128 (partition), b=feat//128 (5 blocks, zero-padded to 640),
    j=token-in-macro. One DMA per macro, no on-device transposes.
  - trunk zT [256, tok] via 2x5 accumulating matmuls -> exact GELU on
    ACT (trunk bias fused as per-partition ACT bias), zs in bf16.
  - host writes a CONSTANT-1.0 row into padded row 96 of feature block 4
    (trunk weights there are zero): all biases ride ordinary K=128
    matmuls against that row -- wrB4 (row96=router_b) for the router,
    whB4 (row96=head biases) for the heads. No K=1 matmuls (their
    LDWEIGHTS is serial/non-FWL and costs ~107ns unhidden).
  - router computed TOKEN-major: per 128-token subtile, 5 matmuls with
    lhsT = xT k-block slice and rhs = Wr block -> logits [128tok,7] PSUM.
  - softmax via tanh identity exp(l) = (1+tanh(l/2))/(1-tanh(l/2)) so
    GELU and softmax share one ACT table set. Raw exps e_p go to slots
    0..6 of e8, their sum S to slot 7; pw8 = e8 * (1/S) gives normalized
    weights with a 1.0 automatically in slot 7 (base head).
  - heads: pe4 [128tok, 4 sub, 320] single 4-bank PSUM tile; columns
    c=(h*20+k)*8+p, p in 0..6 = experts, p=7 = base head. Combine =
    ONE broadcast multiply by pw8 (DVE) + p-reduction split across DVE
    (heads 0:20) and the Pool engine (heads 20:40, pairwise add-tree).
  - queue discipline: x-loads own the Sync queue; weights ride GpSimd/
    Scalar at startup; output DMA issues on the Pool queue right after
    the add-tree so no queue ever blocks behind a not-yet-ready DMA.
  - 8 dependency-free warm-up matmuls at t=0 hold the PE's HAM activity
    window open during the initial transfers so real work starts at
    2.4 GHz instead of the cold 1.2 GHz.
"""

import numpy as np

B, T = 32, 2048
HID, ACTD, SRC, AGE = 512, 64, 2, 16
TRUNK, BINS, PHASES = 256, 20, 7
IN_DIM = HID + ACTD + SRC + AGE  # 594
NCORES = 8
B_LOC = B // NCORES  # 4
NTOK = B_LOC * T  # 8192
MACRO = 512
NMACRO = NTOK // MACRO  # 16
SUB = MACRO // 128  # 4
NBLK = 5  # 594 features zero-padded to 5 k-blocks of 128
NHK = 2 * BINS  # 40 (head, bin) pairs
NP8 = PHASES + 1  # 7 experts + 1 base slot
NCOL = NHK * NP8  # 320 head-matmul output columns

NWARM = 0
_BUILT = {}
LAST_RESULT = None


def _build_module():
    """Build the Bass module (same NEFF for all cores)."""
    import concourse.bass as bass
    import concourse.tile as tile
    from concourse import bacc, mybir

    f32 = mybir.dt.float32
    bf16 = mybir.dt.bfloat16
    AF = mybir.ActivationFunctionType
    ALU = mybir.AluOpType
    ts = bass.ts

    nc = bacc.Bacc("TRN2", target_bir_lowering=False, debug=False)

    x_d = nc.dram_tensor("x", [NMACRO, 128, NBLK, MACRO], bf16, kind="ExternalInput")
    wt_d = nc.dram_tensor("wt", [128, NBLK, TRUNK], bf16, kind="ExternalInput")
    wr_d = nc.dram_tensor("wr", [128, NBLK, PHASES], bf16, kind="ExternalInput")
    wh_d = nc.dram_tensor("wh", [128, 3, NCOL], bf16, kind="ExternalInput")
    tb_d = nc.dram_tensor("tb", [128, 2], f32, kind="ExternalInput")
    out_d = nc.dram_tensor("out", [NTOK, NHK], f32, kind="ExternalOutput")

    ov = out_d[:, :].rearrange("(m s p) hk -> m p s hk", p=128, s=SUB)

    with tile.TileContext(nc) as tc:
        with (
            tc.tile_pool(name="const", bufs=1) as const,
            tc.tile_pool(name="xin", bufs=4) as xin,
            tc.tile_pool(name="zs", bufs=2) as zsp,
            tc.tile_pool(name="sm", bufs=2) as smp,
            tc.tile_pool(name="prod", bufs=1) as prodp,
            tc.tile_pool(name="outp", bufs=3) as outp,
            tc.tile_pool(name="ps_z", bufs=2, space="PSUM") as ps_z,
            tc.tile_pool(name="ps_pw", bufs=1, space="PSUM") as ps_pw,
            tc.tile_pool(name="ps_e", bufs=1, space="PSUM") as ps_e,
        ):
            wt = const.tile([128, NBLK, TRUNK], bf16)
            nc.gpsimd.dma_start(wt, wt_d[:])
            wr = const.tile([128, NBLK, PHASES], bf16)
            nc.scalar.dma_start(wr, wr_d[:])
            wh = const.tile([128, 3, NCOL], bf16)
            nc.scalar.dma_start(wh, wh_d[:])
            tb = const.tile([128, 2], f32)
            nc.scalar.dma_start(tb, tb_d[:])

            pdum = ps_e.tile([128, SUB, MACRO], f32, tag="pe4")
            # HAM warm-up: dummy matmuls with no DMA dependency keep the
            # PE busy through its 3.4us activity window during the initial
            # weight/x transfers, so the real work starts at 2.4 GHz.
            wu = const.tile([128, MACRO], bf16)
            nc.vector.memset(wu, 0.0)
            for _ in range(NWARM):
                nc.tensor.matmul(
                    pdum[:, 0, :], wu[:, 0:128], wu, start=True, stop=True
                )

            # PE prewarm: consume each const via a dummy matmul so later
            # real PE instructions never need a startup semaphore wait.
            nc.tensor.matmul(
                pdum[:, 0, 0:128], wt[:, 0, 0:128], wt[:, 0, 0:128],
                start=True, stop=True,
            )
            nc.tensor.matmul(
                pdum[:PHASES, 0, 0:128], wr[:, 0, :], wt[:, 0, 0:128],
                start=True, stop=True,
            )
            nc.tensor.matmul(
                pdum[:, 1, 0:NCOL], wh[:, 0, 0:128], wh[:, 2, :],
                start=True, stop=True,
            )

            for m in range(NMACRO):
                # ---- load xT (feature-major; two DMAs per macro so the
                # trunk can start once blocks 0-2 land, while keeping the
                # per-dma_start issue cost on the Sync queue low) ----
                xt = xin.tile([128, NBLK, MACRO], bf16)
                if m == 0:
                    # per-block loads so the very first trunk matmul only
                    # waits for 128 KB
                    for b in range(3):
                        nc.sync.dma_start(xt[:, b, :], x_d[m, :, b, :])
                else:
                    nc.sync.dma_start(xt[:, 0:3, :], x_d[m, :, 0:3, :])
                nc.sync.dma_start(xt[:, 3:5, :], x_d[m, :, 3:5, :])

                # ---- trunk + router woven: the 20 tiny router matmuls
                # (token-major, one per (subtile, k-block); block 4 carries
                # the router bias via the host-injected ones row) slot in
                # between the long trunk matmuls so their LDWEIGHTS load
                # during trunk streaming.  Trunk blocks 3,4 come last in
                # both chains so the second x DMA has extra time. ----
                pz0 = ps_z.tile([128, MACRO], f32, tag="pz")
                pz1 = ps_z.tile([128, MACRO], f32, tag="pz")
                zs = zsp.tile([128, 2, MACRO], bf16)
                # router logits: two PSUM banks; subtile s -> bank s//2,
                # column slot s%2.  Groups in different banks interleave
                # (bank-bit clears are independent); groups sharing a bank
                # run sequentially.
                ppw = ps_pw.tile([128, 2, MACRO], f32, tag="ppw")
                trunk_mms = [
                    (pz, col, b)
                    for b in range(NBLK)
                    for (pz, col) in ((pz0, 0), (pz1, 128))
                ]
                # (s0,s2) woven b-synchronous (different banks), then
                # (s1,s3): consecutive router matmuls alternate banks, and
                # groups sharing a bank stay sequential (a group's
                # start=True clears its whole bank's has_written bits,
                # which is safe only for FINISHED groups).
                router_mms = [
                    (s, b)
                    for s01 in (0, 1)
                    for b in range(NBLK)
                    for s in (s01, s01 + 2)
                ]
                for i, (pz, col, b) in enumerate(trunk_mms):
                    nc.tensor.matmul(
                        pz, wt[:, b, col : col + 128], xt[:, b, :],
                        start=(b == 0), stop=(b == 4),
                    )
                    for s, rb_ in router_mms[2 * i : 2 * i + 2]:
                        c0 = (s % 2) * NP8
                        nc.tensor.matmul(
                            ppw[:, s // 2, c0 : c0 + PHASES],
                            xt[:, rb_, ts(s, 128)], wr[:, rb_, :],
                            start=(rb_ == 0), stop=(rb_ == NBLK - 1),
                        )
                    if pz is pz0 and b == 4:
                        nc.scalar.activation(
                            out=zs[:, 0, :], in_=pz0, func=AF.Gelu,
                            bias=tb[:, 0:1], scale=1.0,
                        )
                nc.scalar.activation(
                    out=zs[:, 1, :], in_=pz1, func=AF.Gelu,
                    bias=tb[:, 1:2], scale=1.0,
                )


                # ---- heads: bias rides the block-4 ones row; bias matmuls
                # first (they only need xt), z matmuls after, so the PE
                # keeps busy while the GELUs finish ----
                osb = outp.tile([128, SUB, NHK], f32)
                pe4 = ps_e.tile([128, SUB, MACRO], f32, tag="pe4")
                for s in range(SUB):
                    nc.tensor.matmul(
                        pe4[:, s, 0:NCOL],
                        xt[:, 4, ts(s, 128)], wh[:, 2, :],
                        start=True, stop=False,
                    )
                last = m == NMACRO - 1
                if not last:
                    for s in range(SUB):
                        nc.tensor.matmul(
                            pe4[:, s, 0:NCOL],
                            zs[:, 0, ts(s, 128)], wh[:, 0, :],
                            start=False, stop=False,
                        )
                    for s in range(SUB):
                        nc.tensor.matmul(
                            pe4[:, s, 0:NCOL],
                            zs[:, 1, ts(s, 128)], wh[:, 1, :],
                            start=False, stop=True,
                        )

                # ---- softmax via tanh; pw8 with 1.0 in slot 7 ----
                th = smp.tile([128, SUB, PHASES], f32, tag="th")
                ppw_v = ppw[:, :, 0 : 2 * NP8].rearrange(
                    "p b (s q) -> p b s q", q=NP8
                )
                nc.scalar.activation(
                    out=th, in_=ppw_v[:, :, :, 0:PHASES], func=AF.Tanh,
                    scale=0.5,
                )
                den = smp.tile([128, SUB, PHASES], f32, tag="den")
                # den = 1 - t
                nc.vector.tensor_scalar(
                    out=den, in0=th, scalar1=-1.0, scalar2=1.0,
                    op0=ALU.mult, op1=ALU.add,
                )
                nc.vector.reciprocal_approx_fast(out=den, in_=den)
                e8 = smp.tile([128, SUB, NP8], f32, tag="e8")
                # e_p = (1 + t) / (1 - t) = exp(l_p); S = sum_p e_p -> slot 7
                nc.vector.scalar_tensor_tensor(
                    out=e8[:, :, :PHASES], in0=th, scalar=1.0, in1=den,
                    op0=ALU.add, op1=ALU.mult,
                )
                nc.vector.reduce_sum(
                    out=e8[:, :, PHASES], in_=e8[:, :, :PHASES],
                    axis=mybir.AxisListType.X,
                )
                recS = smp.tile([128, SUB], f32, tag="recS")
                nc.vector.reciprocal_approx_fast(out=recS, in_=e8[:, :, PHASES])
                pw8 = smp.tile([128, SUB, NP8], f32, tag="pw8")
                nc.vector.tensor_tensor(
                    out=pw8, in0=e8,
                    in1=recS[:, :, None].to_broadcast([128, SUB, NP8]),
                    op=ALU.mult,
                )

                # ---- combine (multiply by pw8 on DVE; p-reduction split:
                # DVE reduces heads 0:20, Pool reduces 20:40 via a
                # pairwise add-tree) ----
                def combine(lo, n):
                    prod = prodp.tile([128, n, NHK, NP8], f32, tag="prod")
                    nc.vector.tensor_tensor(
                        out=prod,
                        in0=pe4[:, lo : lo + n, 0:NCOL].rearrange(
                            "p s (hk e) -> p s hk e", e=NP8
                        ),
                        in1=pw8[:, lo : lo + n, None, :].to_broadcast(
                            [128, n, NHK, NP8]
                        ),
                        op=ALU.mult,
                    )
                    nc.vector.reduce_sum(
                        out=osb[:, lo : lo + n, 0:BINS],
                        in_=prod[:, :, 0:BINS, :],
                        axis=mybir.AxisListType.X,
                    )
                    h4 = prodp.tile([128, n, BINS, 4], f32, tag="h4")
                    nc.gpsimd.tensor_tensor(
                        out=h4, in0=prod[:, :, BINS:, 0:4],
                        in1=prod[:, :, BINS:, 4:8], op=ALU.add,
                    )
                    h2 = prodp.tile([128, n, BINS, 2], f32, tag="h2")
                    nc.gpsimd.tensor_tensor(
                        out=h2, in0=h4[:, :, :, 0:2], in1=h4[:, :, :, 2:4],
                        op=ALU.add,
                    )
                    nc.gpsimd.tensor_tensor(
                        out=osb[:, lo : lo + n, BINS:], in0=h2[:, :, :, 0],
                        in1=h2[:, :, :, 1], op=ALU.add,
                    )

                if last:
                    for s in range(SUB):
                        nc.tensor.matmul(
                            pe4[:, s, 0:NCOL],
                            zs[:, 0, ts(s, 128)], wh[:, 0, :],
                            start=False, stop=False,
                        )
                    for s in range(SUB):
                        nc.tensor.matmul(
                            pe4[:, s, 0:NCOL],
                            zs[:, 1, ts(s, 128)], wh[:, 1, :],
                            start=False, stop=True,
                        )
                combine(0, SUB)
                # output DMA on the same queue that produced osb: the
                # issue's wait is satisfied by FIFO order, so it never
                # blocks and no other queue stalls behind it
                nc.gpsimd.dma_start(ov[m], osb)

    nc.compile()
    return nc


def _host_weights(inp):
    """Rearrange weights into on-device layouts (host-side, one-time)."""
    import ml_dtypes

    bf16 = ml_dtypes.bfloat16
    f = np.float32

    wt = np.zeros((128, NBLK, TRUNK), f)
    for b in range(4):
        wt[:, b, :] = inp["trunk_w"][b * 128 : (b + 1) * 128]
    wt[:82, 4, :] = inp["trunk_w"][512:IN_DIM]

    # block 4 of wr carries the router bias against the ones row (k=96)
    wr = np.zeros((128, NBLK, PHASES), f)
    for b in range(4):
        wr[:, b, :] = inp["router_w"][b * 128 : (b + 1) * 128]
    wr[96, 4, :] = inp["router_b"]

    # heads: col c = (h*20+k)*8 + p ; p<7 experts, p=7 base
    wh_full = np.zeros((TRUNK, NHK, NP8), f)
    dr_full = np.zeros((NHK, NP8), f)
    wh_full[:, :BINS, :PHASES] = np.transpose(inp["inst_exp_w"], (1, 2, 0))
    wh_full[:, BINS:, :PHASES] = np.transpose(inp["group_exp_w"], (1, 2, 0))
    wh_full[:, :BINS, PHASES] = inp["inst_base_w"]
    wh_full[:, BINS:, PHASES] = inp["group_base_w"]
    dr_full[:BINS, :PHASES] = inp["inst_exp_b"].T
    dr_full[BINS:, :PHASES] = inp["group_exp_b"].T
    dr_full[:BINS, PHASES] = inp["inst_base_b"]
    dr_full[BINS:, PHASES] = inp["group_base_b"]
    # wh blocks 0,1 = expert/base weights; block 2 = head biases against
    # the ones row (k=96) of xT feature-block 4
    wh = np.zeros((128, 3, NCOL), f)
    wh[:, 0:2, :] = (
        wh_full.reshape(TRUNK, NCOL).reshape(2, 128, NCOL).transpose(1, 0, 2)
    )
    wh[96, 2, :] = dr_full.reshape(NCOL)

    tb = np.ascontiguousarray(inp["trunk_b"].reshape(2, 128).T.astype(f))
    return wt.astype(bf16), wr.astype(bf16), wh.astype(bf16), tb


def kernel(**inputs):
    global LAST_RESULT
    import sys

    if "/opt/trn_rl_repo" not in sys.path:
        sys.path.insert(0, "/opt/trn_rl_repo")
    import ml_dtypes
    from concourse.bass_utils import run_bass_kernel_spmd

    bf16 = ml_dtypes.bfloat16

    inp = {k: np.asarray(v, dtype=np.float32) for k, v in inputs.items()}

    if "nc" not in _BUILT:
        _BUILT["nc"] = _build_module()
    nc = _BUILT["nc"]

    wt, wr, wh, tb = _host_weights(inp)

    x_full = np.concatenate(
        [inp["h_t"], inp["a_t"], inp["d_t"], inp["age_embed"]], axis=-1
    ).astype(bf16)  # [B, T, 594]
    # feature-major transpose + zero-pad 594 -> 640 (5 blocks of 128):
    # xT[c, m, k, b, j] = x[c, m*512 + j, b*128 + k]; padded feature 608
    # (k=96 of block 4) is a constant 1.0 that carries the biases.
    pad = np.zeros((NCORES, NMACRO, MACRO, NBLK * 128 - IN_DIM), bf16)
    pad[:, :, :, 608 - IN_DIM] = 1.0
    x_pad = np.concatenate(
        [x_full.reshape(NCORES, NMACRO, MACRO, IN_DIM), pad], axis=-1
    ).reshape(NCORES, NMACRO, MACRO, NBLK, 128)
    xT = np.ascontiguousarray(np.transpose(x_pad, (0, 1, 4, 3, 2)))

    in_maps = []
    for c in range(NCORES):
        in_maps.append({"x": xT[c], "wt": wt, "wr": wr, "wh": wh, "tb": tb})

    res = run_bass_kernel_spmd(nc, in_maps, core_ids=list(range(NCORES)))
    LAST_RESULT = res

    inst = np.empty((B, T, BINS), np.float32)
    grp = np.empty((B, T, BINS), np.float32)
    for c in range(NCORES):
        o = res.results[c]["out"]
        inst[c * B_LOC : (c + 1) * B_LOC] = o[:, 0:BINS].reshape(B_LOC, T, BINS)
        grp[c * B_LOC : (c + 1) * B_LOC] = o[:, BINS:].reshape(B_LOC, T, BINS)
    return inst, grp
